# revision 16
# baseline (speedup 1.0000x reference)
"""Trainium2 Bass kernel for nn_LiquidLoRALayer (fp16 pipeline).

Computation (forward only; see problem reference):
    hidden <- 3 liquid-dynamics steps on [O, r] state (target = lora_B)
    B_eff   = hidden (the straight-through trick is a numeric no-op)
    out     = (x @ (2*lora_A)^T) @ B_eff^T          # SCALING=2 folded into A

Sharding: data-parallel over the B*S=16384 rows across 8 cores (2048 rows
per core); all small parameters replicated.

The whole pipeline runs in fp16: the kernel is DMA-bound (x in + out out),
so fp16 I/O halves the HBM traffic vs f32 (~33.7MB/core -> ~94us roofline
at 360 GB/s). fp16 keeps ~11 bits of mantissa so the rel-err stays ~1e-3.

Key layout choices:
  * x is pre-packed on the host to [NB, 128, DC*M_BLK] fp16 so each block
    DMA is 32KB-contiguous per partition (max DMA efficiency) and the
    contraction dim d sits on partitions.
  * liquid state is packed [128, OH] (r x o-half stacked on partitions);
    the gate matmuls use 128-deep block-diagonal weights so one matmul
    pair covers both o-halves (half the PE work of a 64-deep split).
  * bulk DMAs ride the two HWDGE queues: x blocks on qSP (sync), outputs
    on qAct (scalar); no SWDGE software descriptors on the hot path.
"""

import numpy as np
from contextlib import ExitStack

# Problem shapes (hardcoded per spec).
B_, S_, D_, O_, R_ = 4, 4096, 4096, 4096, 64
N_CORES = 8
M_TOTAL = B_ * S_
M_CORE = M_TOTAL // N_CORES      # 2048 rows per core
M_BLK = 512
NB = M_CORE // M_BLK             # 4 row blocks per core
DC = D_ // 128                   # 32 contraction chunks
OH = O_ // 2                     # packed-half width
CH = 1024                        # liquid elementwise chunk
NCH = OH // CH

SCALING = 128.0 / 64.0
DT_STEP = 0.1
TAU_MIN = 0.1
TAU_MAX = 10.0
ADAPT_STEPS = 3

PW = 512 + 2 * OH + DC * R_      # param blob cols: 4 W tiles | btp | h0 | at2

LAST_RESULTS = None  # stashed BassKernelResults from the most recent run


def build_nc():
    """Build the per-core Bass program. All 8 cores run this same program
    on different `xt` shards."""
    import concourse.bacc as bacc
    import concourse.tile as tile
    import concourse.mybir as mybir

    f32 = mybir.dt.float32
    f16 = mybir.dt.float16
    AF = mybir.ActivationFunctionType
    ALU = mybir.AluOpType

    nc = bacc.Bacc()
    xt = nc.dram_tensor("xt", [NB, 128, DC * M_BLK], f16, kind="ExternalInput")
    params = nc.dram_tensor("params", [128, PW], f16, kind="ExternalInput")
    sparams = nc.dram_tensor("sparams", [128, 2], f32, kind="ExternalInput")
    out = nc.dram_tensor("out", [M_CORE, O_], f16, kind="ExternalOutput")

    # out rows grouped in pairs of 128-row subtiles: row = n*256 + t*128 + p
    outv = out[:, :].rearrange("(n t p) o -> n p t o", p=128, t=2)

    with tile.TileContext(nc) as tc, ExitStack() as ctx:
        const = ctx.enter_context(tc.tile_pool(name="const", bufs=1))
        lqp = ctx.enter_context(tc.tile_pool(name="lq", bufs=6))
        hpool = ctx.enter_context(tc.tile_pool(name="hbuf", bufs=2))
        xtp = ctx.enter_context(tc.tile_pool(name="xtp", bufs=3))
        outp = ctx.enter_context(tc.tile_pool(name="outp", bufs=2))
        scr = ctx.enter_context(tc.tile_pool(name="scr", bufs=4))
        ps_pre = ctx.enter_context(tc.tile_pool(name="ps_pre", bufs=2, space="PSUM"))
        ps_tt = ctx.enter_context(tc.tile_pool(name="ps_tt", bufs=2, space="PSUM"))
        ps_out = ctx.enter_context(tc.tile_pool(name="ps_out", bufs=2, space="PSUM"))

        def absorb_s(ap):
            t = scr.tile([1, 8], f16, tag="scr_s")
            nc.scalar.copy(out=t[:, 0:1], in_=ap)

        # ---- params: tiny biases then the fp16 blob, heading qAct so the
        # liquid phase can start ~11us in; x blocks are split across BOTH
        # HWDGE queues (b0/b2/b3 on qSP, b1 on qAct) so all input lands by
        # ~41us instead of serializing behind the params on one ring.
        spa = const.tile([128, 2], f32)
        nc.scalar.dma_start(out=spa, in_=sparams[:, :])
        bgd_ap = spa[:, 0:1]
        btd_ap = spa[:, 1:2]

        pa = const.tile([128, PW], f16)
        nc.scalar.dma_start(out=pa, in_=params[:, :])
        w_gt = pa[:, 0:128]          # block-diag target->gate
        w_gh = pa[:, 128:256]        # block-diag h->gate
        w_tt = pa[:, 256:384]        # block-diag target->tau
        w_th = pa[:, 384:512]        # block-diag h->tau
        btp = pa[:, 512:512 + OH]               # packed lora_B^T
        h0 = pa[:, 512 + OH:512 + 2 * OH]       # packed hidden_B^T
        at2 = pa[:, 512 + 2 * OH:PW]            # packed (2*lora_A)^T

        tmin_sb = const.tile([128, 1], f32)
        nc.vector.memset(tmin_sb, TAU_MIN)

        # Pre-warm the Sigmoid/Exp activation tables while the param DMAs
        # are in flight: an ACT_TABLE_LOAD is 1.28us, and the liquid phase
        # only uses these two functions so the tables then stay resident.
        warm = scr.tile([1, 8], f32, tag="scr_w")
        nc.scalar.activation(out=warm[:, 0:1], in_=tmin_sb[0:1, 0:1],
                             func=AF.Sigmoid)
        nc.scalar.activation(out=warm[:, 1:2], in_=tmin_sb[0:1, 0:1],
                             func=AF.Exp)

        tt_all = const.tile([64, M_CORE], f16)   # stage-1 results
        beff = const.tile([64, O_], f16)         # unpacked B_eff^T

        # ---- liquid dynamics (replicated on every core) ---------------------
        # Packed [128, OH]: p<64 -> (r=p, o<OH), p>=64 -> (r=p-64, o>=OH).
        # Block-diagonal weights make each gate matmul 128-deep so one
        # matmul pair (target-part + h-part) covers both o-halves.
        # The 1/tau and 1/a reciprocals must run in f32 (DVE approx recip is
        # fp32-only; TT divide is not valid ISA), the rest of the chain is
        # fp16. TT inputs must share a dtype, so the f32->fp16 handoff is a
        # cast copy (ra16) and sigma(f) is materialized in both precisions.
        def liquid_step(step, h_cur):
            h_new = hpool.tile([128, OH], f16, tag="h", name=f"h{step}")
            for ch in range(NCH):
                csl = slice(ch * CH, (ch + 1) * CH)
                sf32 = lqp.tile([128, CH], f32, tag="lq32", name=f"sf32_{step}_{ch}")
                sf16 = lqp.tile([128, CH], f16, tag="lq16", name=f"sf16_{step}_{ch}")
                s_u = lqp.tile([128, CH], f32, tag="lq32", name=f"su{step}_{ch}")
                for wi, (w_t, w_h, b_ap) in enumerate((
                    (w_gt, w_gh, bgd_ap),
                    (w_tt, w_th, btd_ap),
                )):
                    for j in range(CH // 512):
                        jsl = slice(ch * CH + j * 512, ch * CH + (j + 1) * 512)
                        osl = slice(j * 512, (j + 1) * 512)
                        pre = ps_pre.tile([128, 512], f32, tag="pre",
                                          name=f"pre{step}_{ch}_{wi}_{j}")
                        nc.tensor.matmul(pre, lhsT=w_t, rhs=btp[:, jsl],
                                         start=True, stop=False)
                        nc.tensor.matmul(pre, lhsT=w_h, rhs=h_cur[:, jsl],
                                         start=False, stop=True)
                        if wi == 0:
                            nc.scalar.activation(out=sf32[:, osl], in_=pre,
                                                 func=AF.Sigmoid, bias=b_ap)
                            nc.scalar.activation(out=sf16[:, osl], in_=pre,
                                                 func=AF.Sigmoid, bias=b_ap)
                        else:
                            nc.scalar.activation(out=s_u[:, osl], in_=pre,
                                                 func=AF.Sigmoid, bias=b_ap)
                tau = lqp.tile([128, CH], f32, tag="lq32", name=f"tau{step}_{ch}")
                nc.vector.tensor_scalar(tau, s_u, TAU_MAX - TAU_MIN, TAU_MIN,
                                        ALU.mult, ALU.add)
                rt = lqp.tile([128, CH], f32, tag="lq32", name=f"rt{step}_{ch}")
                nc.vector.reciprocal_approx_fast(out=rt, in_=tau)
                a = lqp.tile([128, CH], f32, tag="lq32", name=f"a{step}_{ch}")
                nc.vector.tensor_add(a, rt, sf32)
                e = lqp.tile([128, CH], f16, tag="lq16", name=f"e{step}_{ch}")
                nc.scalar.activation(out=e, in_=a, func=AF.Exp, scale=-DT_STEP)
                g = lqp.tile([128, CH], f16, tag="lq16", name=f"g{step}_{ch}")
                nc.gpsimd.tensor_mul(g, sf16, btp[:, csl])
                ra = lqp.tile([128, CH], f32, tag="lq32", name=f"ra{step}_{ch}")
                nc.vector.reciprocal_approx_fast(out=ra, in_=a)
                ra16 = lqp.tile([128, CH], f16, tag="lq16", name=f"ra16_{step}_{ch}")
                nc.vector.tensor_copy(out=ra16, in_=ra)
                p_ = lqp.tile([128, CH], f16, tag="lq16", name=f"p{step}_{ch}")
                nc.vector.tensor_mul(p_, ra16, g)
                d_ = lqp.tile([128, CH], f16, tag="lq16", name=f"d{step}_{ch}")
                nc.vector.tensor_sub(d_, h_cur[:, csl], p_)
                de = lqp.tile([128, CH], f16, tag="lq16", name=f"de{step}_{ch}")
                nc.vector.tensor_mul(de, d_, e)
                nc.vector.tensor_add(h_new[:, csl], de, p_)
            return h_new

        # ---- main pipeline stage 1: tt = (2A) @ x_blk^T ---------------------
        xt_sb = {}

        def in_dma(b):
            t = xtp.tile([128, DC * M_BLK], f16, tag="xt", name=f"xt{b}")
            eng = nc.scalar if b == 1 else nc.sync
            eng.dma_start(out=t, in_=xt[b, :, :])
            xt_sb[b] = t

        def in_mm(b):
            t = xt_sb[b]
            tp = ps_tt.tile([64, 512], f32, tag="tt", name=f"ttps{b}")
            for c in range(DC):
                nc.tensor.matmul(
                    tp, lhsT=at2[:, c * R_:(c + 1) * R_],
                    rhs=t[:, c * M_BLK:(c + 1) * M_BLK],
                    start=(c == 0), stop=(c == DC - 1))
            # DVE (not ACT) so the liquid phase's ACT queue stays pure
            # sigmoid/exp (no activation-table thrash)
            nc.vector.tensor_copy(out=tt_all[:, b * M_BLK:(b + 1) * M_BLK],
                                  in_=tp)

        # ---- main pipeline stage 2: out = tt^T @ B_eff^T --------------------
        osb_cur = [None]

        def out_chain(b):
            for ms in range(M_BLK // 128):
                msg = b * (M_BLK // 128) + ms
                pair, tpos = divmod(msg, 2)
                if tpos == 0:
                    osb_cur[0] = outp.tile([128, 2, O_], f16, tag="osb",
                                           name=f"osb{pair}")
                o_sb = osb_cur[0]
                lhsT = tt_all[:, msg * 128:(msg + 1) * 128]
                for oc2 in range(O_ // 1024):
                    op = ps_out.tile([128, 1024], f32, tag="op",
                                     name=f"op{msg}_{oc2}")
                    nc.tensor.matmul(
                        op[:, 0:512], lhsT=lhsT,
                        rhs=beff[:, oc2 * 1024:oc2 * 1024 + 512],
                        start=True, stop=True)
                    nc.tensor.matmul(
                        op[:, 512:1024], lhsT=lhsT,
                        rhs=beff[:, oc2 * 1024 + 512:(oc2 + 1) * 1024],
                        start=True, stop=True)
                    dst = o_sb[:, tpos, oc2 * 1024:(oc2 + 1) * 1024]
                    # ~40% of evictions on DVE, ~60% on ACT: DVE also carries
                    # the liquid chain, ACT is cheaper per element (0.83 vs
                    # 1.04 ns)
                    if (msg * 4 + oc2) % 16 < 7:
                        nc.vector.tensor_copy(out=dst, in_=op)
                    else:
                        nc.scalar.copy(out=dst, in_=op)
                if tpos == 1:
                    nc.scalar.dma_start(out=outv[pair], in_=o_sb)

        # ---- driver ---------------------------------------------------------
        # Emission order sets each engine's program order. Arrival times:
        # params ~11.5us (qAct), b0 ~18 (qSP), b1 ~23 (qAct), b2 ~29 (qSP),
        # b3 ~41 (qSP, WAR on b0's buffer). The PE queue interleaves liquid
        # steps with stage-1 blocks, then stage-2 as soon as beff is ready,
        # so most matmuls run before the DVFS throttle kicks in.
        in_dma(0)
        in_dma(1)
        in_dma(2)
        h = h0
        h = liquid_step(0, h)
        in_mm(0)
        h = liquid_step(1, h)
        in_mm(1)
        h = liquid_step(2, h)

        # unpack B_eff^T to [64, O]
        nc.vector.tensor_copy(out=beff[:, 0:OH], in_=h[0:64, :])
        nc.gpsimd.dma_start(out=beff[:, OH:O_], in_=h[64:128, :])
        absorb_s(beff[0:1, OH:OH + 1])

        in_mm(2)
        in_dma(3)
        out_chain(0)
        in_mm(3)
        out_chain(1)
        out_chain(2)
        out_chain(3)
    nc.finalize()
    return nc


def make_host_inputs(x, lora_A, lora_B, hidden_B, W_gate, b_gate, W_tau,
                     b_tau, n_cores=N_CORES):
    """Host-side sharding / layout prep. Returns the per-core in_maps."""
    f16 = np.float16
    x = np.asarray(x, np.float32).reshape(M_TOTAL, D_)

    WgT = np.asarray(W_gate, np.float32).T                   # [2r, r]
    WtT = np.asarray(W_tau, np.float32).T

    def blockdiag(w64):
        bd = np.zeros((128, 128), np.float32)
        bd[0:64, 0:64] = w64
        bd[64:128, 64:128] = w64
        return bd

    w_gt = blockdiag(WgT[:R_])
    w_gh = blockdiag(WgT[R_:])
    w_tt = blockdiag(WtT[:R_])
    w_th = blockdiag(WtT[R_:])

    BT = np.asarray(lora_B, np.float32).T                    # [r, O]
    btp = np.concatenate([BT[:, :OH], BT[:, OH:]], axis=0)   # [128, OH]
    hT = np.asarray(hidden_B, np.float32).T
    h0p = np.concatenate([hT[:, :OH], hT[:, OH:]], axis=0)
    at2 = (2.0 * np.asarray(lora_A, np.float32)).T           # [D, r]
    at2_pk = at2.reshape(DC, 128, R_).transpose(1, 0, 2).reshape(128, DC * R_)

    params_np = np.ascontiguousarray(np.concatenate(
        [w_gt, w_gh, w_tt, w_th, btp, h0p, at2_pk], axis=1).astype(f16))
    bg = np.asarray(b_gate, np.float32)
    bt = np.asarray(b_tau, np.float32)
    sparams_np = np.ascontiguousarray(
        np.stack([np.concatenate([bg, bg]), np.concatenate([bt, bt])],
                 axis=1).astype(np.float32))

    x16 = x.astype(f16)
    shared = dict(params=params_np, sparams=sparams_np)
    in_maps = []
    for c in range(n_cores):
        xc = x16[c * M_CORE:(c + 1) * M_CORE]
        xp = xc.reshape(NB, M_BLK, DC, 128).transpose(0, 3, 2, 1)
        m = dict(shared)
        m["xt"] = np.ascontiguousarray(xp.reshape(NB, 128, DC * M_BLK))
        in_maps.append(m)
    return in_maps


_NC_CACHE = {}


def kernel(x, lora_A, lora_B, hidden_B, W_gate, b_gate, W_tau, b_tau):
    from concourse.bass_utils import run_bass_kernel_spmd

    global LAST_RESULTS
    key = "main"
    if key not in _NC_CACHE:
        _NC_CACHE[key] = build_nc()
    nc = _NC_CACHE[key]

    in_maps = make_host_inputs(x, lora_A, lora_B, hidden_B,
                               W_gate, b_gate, W_tau, b_tau)
    res = run_bass_kernel_spmd(nc, in_maps, core_ids=list(range(N_CORES)))
    LAST_RESULTS = res
    outs = [np.asarray(res.results[c]["out"]) for c in range(N_CORES)]
    full = np.concatenate(outs, axis=0).astype(np.float32).reshape(B_, S_, O_)
    return np.ascontiguousarray(full)


# revision 21
# speedup vs baseline: 1.1221x; 1.1221x over previous
"""Trainium2 Bass kernel for nn_LiquidLoRALayer (fp16 pipeline).

Computation (forward only; see problem reference):
    hidden <- 3 liquid-dynamics steps on [O, r] state (target = lora_B)
    B_eff   = hidden (the straight-through trick is a numeric no-op)
    out     = (x @ (2*lora_A)^T) @ B_eff^T          # SCALING=2 folded into A

Sharding: data-parallel over the B*S=16384 rows across 8 cores (2048 rows
per core); all small parameters replicated.

The whole pipeline runs in fp16: the kernel is DMA-bound (x in + out out),
so fp16 I/O halves the HBM traffic vs f32 (~33.7MB/core -> ~94us roofline
at 360 GB/s). fp16 keeps ~11 bits of mantissa so the rel-err stays ~1e-3.

Key layout choices:
  * x is pre-packed on the host to [NB, 128, DC*M_BLK] fp16 so each block
    DMA is 32KB-contiguous per partition (max DMA efficiency) and the
    contraction dim d sits on partitions.
  * liquid state is packed [128, OH] (r x o-half stacked on partitions);
    the gate matmuls use 128-deep block-diagonal weights so one matmul
    pair covers both o-halves (half the PE work of a 64-deep split).
  * bulk DMAs ride the two HWDGE queues: x blocks on qSP (sync), outputs
    on qAct (scalar); no SWDGE software descriptors on the hot path.
"""

import numpy as np
from contextlib import ExitStack

# Problem shapes (hardcoded per spec).
B_, S_, D_, O_, R_ = 4, 4096, 4096, 4096, 64
N_CORES = 8
M_TOTAL = B_ * S_
M_CORE = M_TOTAL // N_CORES      # 2048 rows per core
M_BLK = 512
NB = M_CORE // M_BLK             # 4 row blocks per core
DC = D_ // 128                   # 32 contraction chunks
OH = O_ // 2                     # packed-half width
CH = 1024                        # liquid elementwise chunk
NCH = OH // CH

SCALING = 128.0 / 64.0
DT_STEP = 0.1
TAU_MIN = 0.1
TAU_MAX = 10.0
ADAPT_STEPS = 3

PW = 512 + 2 * OH + DC * R_      # param blob cols: 4 W tiles | btp | h0 | at2

LAST_RESULTS = None  # stashed BassKernelResults from the most recent run


def build_nc():
    """Build the per-core Bass program. All 8 cores run this same program
    on different `xt` shards."""
    import concourse.bacc as bacc
    import concourse.tile as tile
    import concourse.mybir as mybir

    f32 = mybir.dt.float32
    f16 = mybir.dt.float16
    AF = mybir.ActivationFunctionType
    ALU = mybir.AluOpType

    nc = bacc.Bacc()
    xt = nc.dram_tensor("xt", [NB, 128, DC * M_BLK], f16, kind="ExternalInput")
    params = nc.dram_tensor("params", [128, PW], f16, kind="ExternalInput")
    sparams = nc.dram_tensor("sparams", [128, 2], f32, kind="ExternalInput")
    out = nc.dram_tensor("out", [M_CORE, O_], f16, kind="ExternalOutput")

    # out rows grouped in pairs of 128-row subtiles: row = n*256 + t*128 + p
    outv = out[:, :].rearrange("(n t p) o -> n p t o", p=128, t=2)

    with tile.TileContext(nc) as tc, ExitStack() as ctx:
        const = ctx.enter_context(tc.tile_pool(name="const", bufs=1))
        lqp = ctx.enter_context(tc.tile_pool(name="lq", bufs=8))
        lq16p = ctx.enter_context(tc.tile_pool(name="lq16", bufs=6))
        gp = ctx.enter_context(tc.tile_pool(name="gp", bufs=2))
        hpool = ctx.enter_context(tc.tile_pool(name="hbuf", bufs=2))
        xtp = ctx.enter_context(tc.tile_pool(name="xtp", bufs=2))
        outp = ctx.enter_context(tc.tile_pool(name="outp", bufs=2))
        scr = ctx.enter_context(tc.tile_pool(name="scr", bufs=4))
        ps_pre = ctx.enter_context(tc.tile_pool(name="ps_pre", bufs=3, space="PSUM"))
        ps_tt = ctx.enter_context(tc.tile_pool(name="ps_tt", bufs=1, space="PSUM"))
        ps_out = ctx.enter_context(tc.tile_pool(name="ps_out", bufs=2, space="PSUM"))

        def absorb_s(ap):
            t = scr.tile([1, 8], f16, tag="scr_s")
            nc.scalar.copy(out=t[:, 0:1], in_=ap)

        # ---- params: tiny biases then the fp16 blob, heading qSP so the
        # liquid phase can start ~11us in; qAct stays free for output DMAs.
        spa = const.tile([128, 2], f32)
        nc.sync.dma_start(out=spa, in_=sparams[:, :])
        bgd_ap = spa[:, 0:1]
        btd_ap = spa[:, 1:2]

        pa = const.tile([128, PW], f16)
        nc.sync.dma_start(out=pa, in_=params[:, :])
        w_gt = pa[:, 0:128]          # block-diag target->gate
        w_gh = pa[:, 128:256]        # block-diag h->gate
        w_tt = pa[:, 256:384]        # block-diag target->tau
        w_th = pa[:, 384:512]        # block-diag h->tau
        btp = pa[:, 512:512 + OH]               # packed lora_B^T
        h0 = pa[:, 512 + OH:512 + 2 * OH]       # packed hidden_B^T
        at2 = pa[:, 512 + 2 * OH:PW]            # packed (2*lora_A)^T

        tmin_sb = const.tile([128, 1], f32)
        nc.vector.memset(tmin_sb, TAU_MIN)

        # f32 copy of packed lora_B^T for the pool-engine gate multiply
        # (TT inputs must share a dtype); converted once on the idle pool.
        btp32 = const.tile([128, OH], f32)
        nc.gpsimd.tensor_copy(out=btp32, in_=btp)

        # Pre-warm the Sigmoid/Exp activation tables while the param DMAs
        # are in flight: an ACT_TABLE_LOAD is 1.28us, and the liquid phase
        # only uses these two functions so the tables then stay resident.
        warm = scr.tile([1, 8], f32, tag="scr_w")
        nc.scalar.activation(out=warm[:, 0:1], in_=tmin_sb[0:1, 0:1],
                             func=AF.Sigmoid)
        nc.scalar.activation(out=warm[:, 1:2], in_=tmin_sb[0:1, 0:1],
                             func=AF.Exp)

        tt_all = const.tile([64, M_CORE], f16)   # stage-1 results
        beff = const.tile([64, O_], f16)         # unpacked B_eff^T

        # ---- liquid dynamics (replicated on every core) ---------------------
        # Packed [128, OH]: p<64 -> (r=p, o<OH), p>=64 -> (r=p-64, o>=OH).
        # Block-diagonal weights make each gate matmul 128-deep so one
        # matmul pair (target-part + h-part) covers both o-halves.
        # The 1/tau and 1/a reciprocals must run in f32 (DVE approx recip is
        # fp32-only; TT divide is not valid ISA), the rest of the chain is
        # fp16. TT inputs must share a dtype, so the f32->fp16 handoff is a
        # cast copy (ra16) and sigma(f) is materialized in both precisions.
        def liquid_step(step, h_cur):
            h_new = hpool.tile([128, OH], f16, tag="h", name=f"h{step}")
            sf = [None] * NCH
            su = [None] * NCH
            gg = [None] * NCH
            aa = [None] * NCH
            ee = [None] * NCH
            # phase 1: all gate matmuls + sigmoids of the step back-to-back
            # (one sigmoid table load), the pool gate-multiply trailing each
            # chunk's sigma(f)
            for ch in range(NCH):
                csl = slice(ch * CH, (ch + 1) * CH)
                sf[ch] = lqp.tile([128, CH], f32, tag="lq32",
                                  name=f"sf{step}_{ch}")
                su[ch] = lqp.tile([128, CH], f32, tag="lq32",
                                  name=f"su{step}_{ch}")
                for wi, (w_t, w_h, b_ap, s_out) in enumerate((
                    (w_gt, w_gh, bgd_ap, sf[ch]),
                    (w_tt, w_th, btd_ap, su[ch]),
                )):
                    for j in range(CH // 512):
                        jsl = slice(ch * CH + j * 512, ch * CH + (j + 1) * 512)
                        osl = slice(j * 512, (j + 1) * 512)
                        pre = ps_pre.tile([128, 512], f32, tag="pre",
                                          name=f"pre{step}_{ch}_{wi}_{j}")
                        nc.tensor.matmul(pre, lhsT=w_t, rhs=btp[:, jsl],
                                         start=True, stop=False)
                        nc.tensor.matmul(pre, lhsT=w_h, rhs=h_cur[:, jsl],
                                         start=False, stop=True)
                        nc.scalar.activation(out=s_out[:, osl], in_=pre,
                                             func=AF.Sigmoid, bias=b_ap)
                gg[ch] = gp.tile([128, CH], f32, tag="g", name=f"g{step}_{ch}")
                nc.gpsimd.tensor_mul(gg[ch], sf[ch], btp32[:, csl])
            # phase 2: tau -> 1/tau -> a per chunk (DVE)
            for ch in range(NCH):
                tau = lqp.tile([128, CH], f32, tag="lq32", name=f"tau{step}_{ch}")
                nc.vector.tensor_scalar(tau, su[ch], TAU_MAX - TAU_MIN,
                                        TAU_MIN, ALU.mult, ALU.add)
                rt = lqp.tile([128, CH], f32, tag="lq32", name=f"rt{step}_{ch}")
                nc.vector.reciprocal_approx_fast(out=rt, in_=tau)
                aa[ch] = lqp.tile([128, CH], f32, tag="lq32",
                                  name=f"a{step}_{ch}")
                nc.vector.tensor_add(aa[ch], rt, sf[ch])
            # phase 3: both exps together (one exp table load)
            for ch in range(NCH):
                ee[ch] = lq16p.tile([128, CH], f16, tag="lq16",
                                  name=f"e{step}_{ch}")
                nc.scalar.activation(out=ee[ch], in_=aa[ch], func=AF.Exp,
                                     scale=-DT_STEP)
            # phase 4: closed-form update h' = p + (h - p)*e per chunk
            for ch in range(NCH):
                csl = slice(ch * CH, (ch + 1) * CH)
                ra = lqp.tile([128, CH], f32, tag="lq32", name=f"ra{step}_{ch}")
                nc.vector.reciprocal_approx_fast(out=ra, in_=aa[ch])
                p32 = lqp.tile([128, CH], f32, tag="lq32", name=f"p32_{step}_{ch}")
                nc.vector.tensor_mul(p32, ra, gg[ch])
                p_ = lq16p.tile([128, CH], f16, tag="lq16", name=f"p{step}_{ch}")
                nc.vector.tensor_copy(out=p_, in_=p32)
                d_ = lq16p.tile([128, CH], f16, tag="lq16", name=f"d{step}_{ch}")
                nc.vector.tensor_sub(d_, h_cur[:, csl], p_)
                de = lq16p.tile([128, CH], f16, tag="lq16", name=f"de{step}_{ch}")
                nc.vector.tensor_mul(de, d_, ee[ch])
                nc.vector.tensor_add(h_new[:, csl], de, p_)
            return h_new

        # ---- main pipeline stage 1: tt = (2A) @ x_blk^T ---------------------
        xt_sb = {}

        def in_dma(b):
            t = xtp.tile([128, DC * M_BLK], f16, tag="xt", name=f"xt{b}")
            nc.sync.dma_start(out=t, in_=xt[b, :, :])
            xt_sb[b] = t

        def in_mm(b):
            t = xt_sb[b]
            tp = ps_tt.tile([64, 512], f32, tag="tt", name=f"ttps{b}")
            for c in range(DC):
                nc.tensor.matmul(
                    tp, lhsT=at2[:, c * R_:(c + 1) * R_],
                    rhs=t[:, c * M_BLK:(c + 1) * M_BLK],
                    start=(c == 0), stop=(c == DC - 1))
            # DVE (not ACT) so the liquid phase's ACT queue stays pure
            # sigmoid/exp (no activation-table thrash)
            nc.vector.tensor_copy(out=tt_all[:, b * M_BLK:(b + 1) * M_BLK],
                                  in_=tp)

        # ---- main pipeline stage 2: out = tt^T @ B_eff^T --------------------
        osb_cur = [None]

        def out_chain(b):
            for ms in range(M_BLK // 128):
                msg = b * (M_BLK // 128) + ms
                pair, tpos = divmod(msg, 2)
                if tpos == 0:
                    osb_cur[0] = outp.tile([128, 2, O_], f16, tag="osb",
                                           name=f"osb{pair}")
                o_sb = osb_cur[0]
                lhsT = tt_all[:, msg * 128:(msg + 1) * 128]
                for oc2 in range(O_ // 1024):
                    op = ps_out.tile([128, 1024], f32, tag="op",
                                     name=f"op{msg}_{oc2}")
                    nc.tensor.matmul(
                        op[:, 0:512], lhsT=lhsT,
                        rhs=beff[:, oc2 * 1024:oc2 * 1024 + 512],
                        start=True, stop=True)
                    nc.tensor.matmul(
                        op[:, 512:1024], lhsT=lhsT,
                        rhs=beff[:, oc2 * 1024 + 512:(oc2 + 1) * 1024],
                        start=True, stop=True)
                    dst = o_sb[:, tpos, oc2 * 1024:(oc2 + 1) * 1024]
                    # strictly alternate DVE/ACT so consecutive PSUM
                    # evictions overlap and never serialize on one engine
                    if (msg * 4 + oc2) % 2 == 0:
                        nc.vector.tensor_copy(out=dst, in_=op)
                    else:
                        nc.scalar.copy(out=dst, in_=op)
                if tpos == 1:
                    nc.scalar.dma_start(out=outv[pair], in_=o_sb)

        # ---- driver ---------------------------------------------------------
        # Emission order sets each engine's program order. Arrival times:
        # params ~11.5us (qAct), b0 ~18 (qSP), b1 ~23 (qAct), b2 ~29 (qSP),
        # b3 ~41 (qSP, WAR on b0's buffer). The PE queue interleaves liquid
        # steps with stage-1 blocks, then stage-2 as soon as beff is ready,
        # so most matmuls run before the DVFS throttle kicks in.
        in_dma(0)
        in_dma(1)
        in_dma(2)
        h = h0
        h = liquid_step(0, h)
        in_mm(0)
        h = liquid_step(1, h)
        in_mm(1)
        h = liquid_step(2, h)

        # unpack B_eff^T to [64, O]
        nc.vector.tensor_copy(out=beff[:, 0:OH], in_=h[0:64, :])
        nc.gpsimd.dma_start(out=beff[:, OH:O_], in_=h[64:128, :])
        absorb_s(beff[0:1, OH:OH + 1])

        in_mm(2)
        in_dma(3)
        out_chain(0)
        in_mm(3)
        out_chain(1)
        out_chain(2)
        out_chain(3)
    nc.finalize()
    return nc


def make_host_inputs(x, lora_A, lora_B, hidden_B, W_gate, b_gate, W_tau,
                     b_tau, n_cores=N_CORES):
    """Host-side sharding / layout prep. Returns the per-core in_maps."""
    f16 = np.float16
    x = np.asarray(x, np.float32).reshape(M_TOTAL, D_)

    WgT = np.asarray(W_gate, np.float32).T                   # [2r, r]
    WtT = np.asarray(W_tau, np.float32).T

    def blockdiag(w64):
        bd = np.zeros((128, 128), np.float32)
        bd[0:64, 0:64] = w64
        bd[64:128, 64:128] = w64
        return bd

    w_gt = blockdiag(WgT[:R_])
    w_gh = blockdiag(WgT[R_:])
    w_tt = blockdiag(WtT[:R_])
    w_th = blockdiag(WtT[R_:])

    BT = np.asarray(lora_B, np.float32).T                    # [r, O]
    btp = np.concatenate([BT[:, :OH], BT[:, OH:]], axis=0)   # [128, OH]
    hT = np.asarray(hidden_B, np.float32).T
    h0p = np.concatenate([hT[:, :OH], hT[:, OH:]], axis=0)
    at2 = (2.0 * np.asarray(lora_A, np.float32)).T           # [D, r]
    at2_pk = at2.reshape(DC, 128, R_).transpose(1, 0, 2).reshape(128, DC * R_)

    params_np = np.ascontiguousarray(np.concatenate(
        [w_gt, w_gh, w_tt, w_th, btp, h0p, at2_pk], axis=1).astype(f16))
    bg = np.asarray(b_gate, np.float32)
    bt = np.asarray(b_tau, np.float32)
    sparams_np = np.ascontiguousarray(
        np.stack([np.concatenate([bg, bg]), np.concatenate([bt, bt])],
                 axis=1).astype(np.float32))

    x16 = x.astype(f16)
    shared = dict(params=params_np, sparams=sparams_np)
    in_maps = []
    for c in range(n_cores):
        xc = x16[c * M_CORE:(c + 1) * M_CORE]
        xp = xc.reshape(NB, M_BLK, DC, 128).transpose(0, 3, 2, 1)
        m = dict(shared)
        m["xt"] = np.ascontiguousarray(xp.reshape(NB, 128, DC * M_BLK))
        in_maps.append(m)
    return in_maps


_NC_CACHE = {}


def kernel(x, lora_A, lora_B, hidden_B, W_gate, b_gate, W_tau, b_tau):
    from concourse.bass_utils import run_bass_kernel_spmd

    global LAST_RESULTS
    key = "main"
    if key not in _NC_CACHE:
        _NC_CACHE[key] = build_nc()
    nc = _NC_CACHE[key]

    in_maps = make_host_inputs(x, lora_A, lora_B, hidden_B,
                               W_gate, b_gate, W_tau, b_tau)
    res = run_bass_kernel_spmd(nc, in_maps, core_ids=list(range(N_CORES)))
    LAST_RESULTS = res
    outs = [np.asarray(res.results[c]["out"]) for c in range(N_CORES)]
    full = np.concatenate(outs, axis=0).astype(np.float32).reshape(B_, S_, O_)
    return np.ascontiguousarray(full)


# revision 22
# speedup vs baseline: 1.1909x; 1.0612x over previous
"""Trainium2 Bass kernel for nn_LiquidLoRALayer (fp16 pipeline).

Computation (forward only; see problem reference):
    hidden <- 3 liquid-dynamics steps on [O, r] state (target = lora_B)
    B_eff   = hidden (the straight-through trick is a numeric no-op)
    out     = (x @ (2*lora_A)^T) @ B_eff^T          # SCALING=2 folded into A

Sharding: data-parallel over the B*S=16384 rows across 8 cores (2048 rows
per core); all small parameters replicated.

The whole pipeline runs in fp16: the kernel is DMA-bound (x in + out out),
so fp16 I/O halves the HBM traffic vs f32 (~33.7MB/core -> ~94us roofline
at 360 GB/s). fp16 keeps ~11 bits of mantissa so the rel-err stays ~1e-3.

Key layout choices:
  * x is pre-packed on the host to [NB, 128, DC*M_BLK] fp16 so each block
    DMA is 32KB-contiguous per partition (max DMA efficiency) and the
    contraction dim d sits on partitions.
  * liquid state is packed [128, OH] (r x o-half stacked on partitions);
    the gate matmuls use 128-deep block-diagonal weights so one matmul
    pair covers both o-halves (half the PE work of a 64-deep split).
  * bulk DMAs ride the two HWDGE queues: x blocks on qSP (sync), outputs
    on qAct (scalar); no SWDGE software descriptors on the hot path.
"""

import numpy as np
from contextlib import ExitStack

# Problem shapes (hardcoded per spec).
B_, S_, D_, O_, R_ = 4, 4096, 4096, 4096, 64
N_CORES = 8
M_TOTAL = B_ * S_
M_CORE = M_TOTAL // N_CORES      # 2048 rows per core
M_BLK = 512
NB = M_CORE // M_BLK             # 4 row blocks per core
DC = D_ // 128                   # 32 contraction chunks
OH = O_ // 2                     # packed-half width
CH = 1024                        # liquid elementwise chunk
NCH = OH // CH

SCALING = 128.0 / 64.0
DT_STEP = 0.1
TAU_MIN = 0.1
TAU_MAX = 10.0
ADAPT_STEPS = 3

PW = 512 + 2 * OH + DC * R_      # param blob cols: 4 W tiles | btp | h0 | at2

LAST_RESULTS = None  # stashed BassKernelResults from the most recent run


def build_nc():
    """Build the per-core Bass program. All 8 cores run this same program
    on different `xt` shards."""
    import concourse.bacc as bacc
    import concourse.tile as tile
    import concourse.mybir as mybir

    f32 = mybir.dt.float32
    f16 = mybir.dt.float16
    AF = mybir.ActivationFunctionType
    ALU = mybir.AluOpType

    nc = bacc.Bacc()
    xt = nc.dram_tensor("xt", [NB, 128, DC * M_BLK], f16, kind="ExternalInput")
    params = nc.dram_tensor("params", [128, PW], f16, kind="ExternalInput")
    sparams = nc.dram_tensor("sparams", [128, 2], f32, kind="ExternalInput")
    out = nc.dram_tensor("out", [M_CORE, O_], f16, kind="ExternalOutput")

    # out rows grouped in pairs of 128-row subtiles: row = n*256 + t*128 + p
    outv = out[:, :].rearrange("(n t p) o -> n p t o", p=128, t=2)

    with tile.TileContext(nc) as tc, ExitStack() as ctx:
        const = ctx.enter_context(tc.tile_pool(name="const", bufs=1))
        lqp = ctx.enter_context(tc.tile_pool(name="lq", bufs=8))
        lq16p = ctx.enter_context(tc.tile_pool(name="lq16", bufs=6))
        gp = ctx.enter_context(tc.tile_pool(name="gp", bufs=2))
        hpool = ctx.enter_context(tc.tile_pool(name="hbuf", bufs=2))
        xtp = ctx.enter_context(tc.tile_pool(name="xtp", bufs=2))
        outp = ctx.enter_context(tc.tile_pool(name="outp", bufs=2))
        scr = ctx.enter_context(tc.tile_pool(name="scr", bufs=4))
        ps_pre = ctx.enter_context(tc.tile_pool(name="ps_pre", bufs=2, space="PSUM"))
        ps_out = ctx.enter_context(tc.tile_pool(name="ps_out", bufs=3, space="PSUM"))

        def absorb_s(ap):
            t = scr.tile([1, 8], f16, tag="scr_s")
            nc.scalar.copy(out=t[:, 0:1], in_=ap)

        # ---- params: tiny biases then the fp16 blob, heading qSP so the
        # liquid phase can start ~11us in; qAct stays free for output DMAs.
        spa = const.tile([128, 2], f32)
        nc.sync.dma_start(out=spa, in_=sparams[:, :])
        bgd_ap = spa[:, 0:1]
        btd_ap = spa[:, 1:2]

        pa = const.tile([128, PW], f16)
        nc.sync.dma_start(out=pa, in_=params[:, :])
        w_gt = pa[:, 0:128]          # block-diag target->gate
        w_gh = pa[:, 128:256]        # block-diag h->gate
        w_tt = pa[:, 256:384]        # block-diag target->tau
        w_th = pa[:, 384:512]        # block-diag h->tau
        btp = pa[:, 512:512 + OH]               # packed lora_B^T
        h0 = pa[:, 512 + OH:512 + 2 * OH]       # packed hidden_B^T
        at2 = pa[:, 512 + 2 * OH:PW]            # packed (2*lora_A)^T

        tmin_sb = const.tile([128, 1], f32)
        nc.vector.memset(tmin_sb, TAU_MIN)

        # Pre-warm the Sigmoid/Exp activation tables while the param DMAs
        # are in flight: an ACT_TABLE_LOAD is 1.28us, and the liquid phase
        # only uses these two functions so the tables then stay resident.
        warm = scr.tile([1, 8], f32, tag="scr_w")
        nc.scalar.activation(out=warm[:, 0:1], in_=tmin_sb[0:1, 0:1],
                             func=AF.Sigmoid)
        nc.scalar.activation(out=warm[:, 1:2], in_=tmin_sb[0:1, 0:1],
                             func=AF.Exp)

        tt_all = const.tile([64, M_CORE], f16)   # stage-1 results
        beff = const.tile([64, O_], f16)         # unpacked B_eff^T

        # ---- liquid dynamics (replicated on every core) ---------------------
        # Packed [128, OH]: p<64 -> (r=p, o<OH), p>=64 -> (r=p-64, o>=OH).
        # Block-diagonal weights make each gate matmul 128-deep so one
        # matmul pair (target-part + h-part) covers both o-halves.
        # The 1/tau and 1/a reciprocals must run in f32 (DVE approx recip is
        # fp32-only; TT divide is not valid ISA), the rest of the chain is
        # fp16. TT inputs must share a dtype, so the f32->fp16 handoff is a
        # cast copy (ra16) and sigma(f) is materialized in both precisions.
        def liquid_step(step, h_cur):
            h_new = hpool.tile([128, OH], f16, tag="h", name=f"h{step}")
            for ch in range(NCH):
                csl = slice(ch * CH, (ch + 1) * CH)
                sf32 = lqp.tile([128, CH], f32, tag="lq32", name=f"sf32_{step}_{ch}")
                sf16 = lq16p.tile([128, CH], f16, tag="lq16", name=f"sf16_{step}_{ch}")
                s_u = lqp.tile([128, CH], f32, tag="lq32", name=f"su{step}_{ch}")
                for wi, (w_t, w_h, b_ap) in enumerate((
                    (w_gt, w_gh, bgd_ap),
                    (w_tt, w_th, btd_ap),
                )):
                    for j in range(CH // 512):
                        jsl = slice(ch * CH + j * 512, ch * CH + (j + 1) * 512)
                        osl = slice(j * 512, (j + 1) * 512)
                        pre = ps_pre.tile([128, 512], f32, tag="pre",
                                          name=f"pre{step}_{ch}_{wi}_{j}")
                        nc.tensor.matmul(pre, lhsT=w_t, rhs=btp[:, jsl],
                                         start=True, stop=False)
                        nc.tensor.matmul(pre, lhsT=w_h, rhs=h_cur[:, jsl],
                                         start=False, stop=True)
                        if wi == 0:
                            nc.scalar.activation(out=sf32[:, osl], in_=pre,
                                                 func=AF.Sigmoid, bias=b_ap)
                            nc.scalar.activation(out=sf16[:, osl], in_=pre,
                                                 func=AF.Sigmoid, bias=b_ap)
                        else:
                            nc.scalar.activation(out=s_u[:, osl], in_=pre,
                                                 func=AF.Sigmoid, bias=b_ap)
                tau = lqp.tile([128, CH], f32, tag="lq32", name=f"tau{step}_{ch}")
                nc.vector.tensor_scalar(tau, s_u, TAU_MAX - TAU_MIN, TAU_MIN,
                                        ALU.mult, ALU.add)
                rt = lqp.tile([128, CH], f32, tag="lq32", name=f"rt{step}_{ch}")
                nc.vector.reciprocal_approx_fast(out=rt, in_=tau)
                a = lqp.tile([128, CH], f32, tag="lq32", name=f"a{step}_{ch}")
                nc.vector.tensor_add(a, rt, sf32)
                e = lq16p.tile([128, CH], f16, tag="lq16", name=f"e{step}_{ch}")
                nc.scalar.activation(out=e, in_=a, func=AF.Exp, scale=-DT_STEP)
                g = gp.tile([128, CH], f16, tag="g", name=f"g{step}_{ch}")
                nc.gpsimd.tensor_mul(g, sf16, btp[:, csl])
                ra = lqp.tile([128, CH], f32, tag="lq32", name=f"ra{step}_{ch}")
                nc.vector.reciprocal_approx_fast(out=ra, in_=a)
                ra16 = lq16p.tile([128, CH], f16, tag="lq16",
                                  name=f"ra16_{step}_{ch}")
                nc.vector.tensor_copy(out=ra16, in_=ra)
                p_ = lq16p.tile([128, CH], f16, tag="lq16", name=f"p{step}_{ch}")
                nc.vector.tensor_mul(p_, ra16, g)
                d_ = lq16p.tile([128, CH], f16, tag="lq16", name=f"d{step}_{ch}")
                nc.vector.tensor_sub(d_, h_cur[:, csl], p_)
                de = lq16p.tile([128, CH], f16, tag="lq16", name=f"de{step}_{ch}")
                nc.vector.tensor_mul(de, d_, e)
                nc.vector.tensor_add(h_new[:, csl], de, p_)
            return h_new

        # ---- main pipeline stage 1: tt = (2A) @ x_blk^T ---------------------
        xt_sb = {}

        def in_dma(b):
            t = xtp.tile([128, DC * M_BLK], f16, tag="xt", name=f"xt{b}")
            nc.sync.dma_start(out=t, in_=xt[b, :, :])
            xt_sb[b] = t

        def in_mm(b):
            t = xt_sb[b]
            tpt = ps_out.tile([128, 1024], f32, tag="op", name=f"ttps{b}")
            tp = tpt[0:64, 0:512]
            for c in range(DC):
                nc.tensor.matmul(
                    tp, lhsT=at2[:, c * R_:(c + 1) * R_],
                    rhs=t[:, c * M_BLK:(c + 1) * M_BLK],
                    start=(c == 0), stop=(c == DC - 1))
            # DVE (not ACT) so the liquid phase's ACT queue stays pure
            # sigmoid/exp (no activation-table thrash)
            nc.vector.tensor_copy(out=tt_all[:, b * M_BLK:(b + 1) * M_BLK],
                                  in_=tp)

        # ---- main pipeline stage 2: out = tt^T @ B_eff^T --------------------
        osb_cur = [None]

        def out_chain(b):
            for ms in range(M_BLK // 128):
                msg = b * (M_BLK // 128) + ms
                pair, tpos = divmod(msg, 2)
                if tpos == 0:
                    osb_cur[0] = outp.tile([128, 2, O_], f16, tag="osb",
                                           name=f"osb{pair}")
                o_sb = osb_cur[0]
                lhsT = tt_all[:, msg * 128:(msg + 1) * 128]
                for oc2 in range(O_ // 1024):
                    op = ps_out.tile([128, 1024], f32, tag="op",
                                     name=f"op{msg}_{oc2}")
                    nc.tensor.matmul(
                        op[:, 0:512], lhsT=lhsT,
                        rhs=beff[:, oc2 * 1024:oc2 * 1024 + 512],
                        start=True, stop=True)
                    nc.tensor.matmul(
                        op[:, 512:1024], lhsT=lhsT,
                        rhs=beff[:, oc2 * 1024 + 512:(oc2 + 1) * 1024],
                        start=True, stop=True)
                    dst = o_sb[:, tpos, oc2 * 1024:(oc2 + 1) * 1024]
                    # strictly alternate DVE/ACT so consecutive PSUM
                    # evictions overlap and never serialize on one engine
                    if (msg * 4 + oc2) % 2 == 0:
                        nc.vector.tensor_copy(out=dst, in_=op)
                    else:
                        nc.scalar.copy(out=dst, in_=op)
                if tpos == 1:
                    nc.scalar.dma_start(out=outv[pair], in_=o_sb)

        # ---- driver ---------------------------------------------------------
        # Emission order sets each engine's program order. Arrival times:
        # params ~11.5us (qAct), b0 ~18 (qSP), b1 ~23 (qAct), b2 ~29 (qSP),
        # b3 ~41 (qSP, WAR on b0's buffer). The PE queue interleaves liquid
        # steps with stage-1 blocks, then stage-2 as soon as beff is ready,
        # so most matmuls run before the DVFS throttle kicks in.
        in_dma(0)
        in_dma(1)
        in_dma(2)
        h = h0
        h = liquid_step(0, h)
        in_mm(0)
        h = liquid_step(1, h)
        in_mm(1)
        h = liquid_step(2, h)

        # unpack B_eff^T to [64, O]
        nc.vector.tensor_copy(out=beff[:, 0:OH], in_=h[0:64, :])
        nc.gpsimd.dma_start(out=beff[:, OH:O_], in_=h[64:128, :])
        absorb_s(beff[0:1, OH:OH + 1])

        in_mm(2)
        in_dma(3)
        out_chain(0)
        in_mm(3)
        out_chain(1)
        out_chain(2)
        out_chain(3)
    nc.finalize()
    return nc


def make_host_inputs(x, lora_A, lora_B, hidden_B, W_gate, b_gate, W_tau,
                     b_tau, n_cores=N_CORES):
    """Host-side sharding / layout prep. Returns the per-core in_maps."""
    f16 = np.float16
    x = np.asarray(x, np.float32).reshape(M_TOTAL, D_)

    WgT = np.asarray(W_gate, np.float32).T                   # [2r, r]
    WtT = np.asarray(W_tau, np.float32).T

    def blockdiag(w64):
        bd = np.zeros((128, 128), np.float32)
        bd[0:64, 0:64] = w64
        bd[64:128, 64:128] = w64
        return bd

    w_gt = blockdiag(WgT[:R_])
    w_gh = blockdiag(WgT[R_:])
    w_tt = blockdiag(WtT[:R_])
    w_th = blockdiag(WtT[R_:])

    BT = np.asarray(lora_B, np.float32).T                    # [r, O]
    btp = np.concatenate([BT[:, :OH], BT[:, OH:]], axis=0)   # [128, OH]
    hT = np.asarray(hidden_B, np.float32).T
    h0p = np.concatenate([hT[:, :OH], hT[:, OH:]], axis=0)
    at2 = (2.0 * np.asarray(lora_A, np.float32)).T           # [D, r]
    at2_pk = at2.reshape(DC, 128, R_).transpose(1, 0, 2).reshape(128, DC * R_)

    params_np = np.ascontiguousarray(np.concatenate(
        [w_gt, w_gh, w_tt, w_th, btp, h0p, at2_pk], axis=1).astype(f16))
    bg = np.asarray(b_gate, np.float32)
    bt = np.asarray(b_tau, np.float32)
    sparams_np = np.ascontiguousarray(
        np.stack([np.concatenate([bg, bg]), np.concatenate([bt, bt])],
                 axis=1).astype(np.float32))

    x16 = x.astype(f16)
    shared = dict(params=params_np, sparams=sparams_np)
    in_maps = []
    for c in range(n_cores):
        xc = x16[c * M_CORE:(c + 1) * M_CORE]
        xp = xc.reshape(NB, M_BLK, DC, 128).transpose(0, 3, 2, 1)
        m = dict(shared)
        m["xt"] = np.ascontiguousarray(xp.reshape(NB, 128, DC * M_BLK))
        in_maps.append(m)
    return in_maps


_NC_CACHE = {}


def kernel(x, lora_A, lora_B, hidden_B, W_gate, b_gate, W_tau, b_tau):
    from concourse.bass_utils import run_bass_kernel_spmd

    global LAST_RESULTS
    key = "main"
    if key not in _NC_CACHE:
        _NC_CACHE[key] = build_nc()
    nc = _NC_CACHE[key]

    in_maps = make_host_inputs(x, lora_A, lora_B, hidden_B,
                               W_gate, b_gate, W_tau, b_tau)
    res = run_bass_kernel_spmd(nc, in_maps, core_ids=list(range(N_CORES)))
    LAST_RESULTS = res
    outs = [np.asarray(res.results[c]["out"]) for c in range(N_CORES)]
    full = np.concatenate(outs, axis=0).astype(np.float32).reshape(B_, S_, O_)
    return np.ascontiguousarray(full)


# revision 24
# speedup vs baseline: 1.2470x; 1.0471x over previous
"""Trainium2 Bass kernel for nn_LiquidLoRALayer (fp16 pipeline).

Computation (forward only; see problem reference):
    hidden <- 3 liquid-dynamics steps on [O, r] state (target = lora_B)
    B_eff   = hidden (the straight-through trick is a numeric no-op)
    out     = (x @ (2*lora_A)^T) @ B_eff^T          # SCALING=2 folded into A

Sharding: data-parallel over the B*S=16384 rows across 8 cores (2048 rows
per core); all small parameters replicated.

The whole pipeline runs in fp16: the kernel is DMA-bound (x in + out out),
so fp16 I/O halves the HBM traffic vs f32 (~33.7MB/core -> ~94us roofline
at 360 GB/s). fp16 keeps ~11 bits of mantissa so the rel-err stays ~1e-3.

Key layout choices:
  * x is pre-packed on the host to [NB, 128, DC*M_BLK] fp16 so each block
    DMA is 32KB-contiguous per partition (max DMA efficiency) and the
    contraction dim d sits on partitions.
  * liquid state is packed [128, OH] (r x o-half stacked on partitions);
    the gate matmuls use 128-deep block-diagonal weights so one matmul
    pair covers both o-halves (half the PE work of a 64-deep split).
  * bulk DMAs ride the two HWDGE queues: x blocks on qSP (sync), outputs
    on qAct (scalar); no SWDGE software descriptors on the hot path.
"""

import numpy as np
from contextlib import ExitStack

# Problem shapes (hardcoded per spec).
B_, S_, D_, O_, R_ = 4, 4096, 4096, 4096, 64
N_CORES = 8
M_TOTAL = B_ * S_
M_CORE = M_TOTAL // N_CORES      # 2048 rows per core
M_BLK = 512
NB = M_CORE // M_BLK             # 4 row blocks per core
DC = D_ // 128                   # 32 contraction chunks
OH = O_ // 2                     # packed-half width
CH = 1024                        # liquid elementwise chunk
NCH = OH // CH

SCALING = 128.0 / 64.0
DT_STEP = 0.1
TAU_MIN = 0.1
TAU_MAX = 10.0
ADAPT_STEPS = 3

PW = 512 + 2 * OH + DC * R_      # param blob cols: 4 W tiles | btp | h0 | at2

LAST_RESULTS = None  # stashed BassKernelResults from the most recent run


def build_nc():
    """Build the per-core Bass program. All 8 cores run this same program
    on different `xt` shards."""
    import concourse.bacc as bacc
    import concourse.tile as tile
    import concourse.mybir as mybir

    f32 = mybir.dt.float32
    f16 = mybir.dt.float16
    AF = mybir.ActivationFunctionType
    ALU = mybir.AluOpType

    nc = bacc.Bacc()
    xt = nc.dram_tensor("xt", [NB, 128, DC * M_BLK], f16, kind="ExternalInput")
    params = nc.dram_tensor("params", [128, PW], f16, kind="ExternalInput")
    sparams = nc.dram_tensor("sparams", [128, 2], f32, kind="ExternalInput")
    out = nc.dram_tensor("out", [M_CORE, O_], f16, kind="ExternalOutput")

    # out rows grouped in pairs of 128-row subtiles: row = n*256 + t*128 + p
    outv = out[:, :].rearrange("(n t p) o -> n p t o", p=128, t=2)

    with tile.TileContext(nc) as tc, ExitStack() as ctx:
        const = ctx.enter_context(tc.tile_pool(name="const", bufs=1))
        lqp = ctx.enter_context(tc.tile_pool(name="lq", bufs=7))
        lq16p = ctx.enter_context(tc.tile_pool(name="lq16", bufs=5))
        gp = ctx.enter_context(tc.tile_pool(name="gp", bufs=2))
        hpool = ctx.enter_context(tc.tile_pool(name="hbuf", bufs=2))
        xtp = ctx.enter_context(tc.tile_pool(name="xtp", bufs=3))
        outp = ctx.enter_context(tc.tile_pool(name="outp", bufs=2))
        scr = ctx.enter_context(tc.tile_pool(name="scr", bufs=4))
        ps_pre = ctx.enter_context(tc.tile_pool(name="ps_pre", bufs=2, space="PSUM"))
        ps_out = ctx.enter_context(tc.tile_pool(name="ps_out", bufs=3, space="PSUM"))

        def absorb_s(ap):
            t = scr.tile([1, 8], f16, tag="scr_s")
            nc.scalar.copy(out=t[:, 0:1], in_=ap)

        # ---- params: tiny biases then the fp16 blob, heading qSP so the
        # liquid phase can start ~11us in; qAct stays free for output DMAs.
        spa = const.tile([128, 2], f32)
        nc.sync.dma_start(out=spa, in_=sparams[:, :])
        bgd_ap = spa[:, 0:1]
        btd_ap = spa[:, 1:2]

        pa = const.tile([128, PW], f16)
        nc.sync.dma_start(out=pa[:, 0:512 + 2 * OH], in_=params[:, 0:512 + 2 * OH])
        nc.sync.dma_start(out=pa[:, 512 + 2 * OH:PW], in_=params[:, 512 + 2 * OH:PW])
        w_gt = pa[:, 0:128]          # block-diag target->gate
        w_gh = pa[:, 128:256]        # block-diag h->gate
        w_tt = pa[:, 256:384]        # block-diag target->tau
        w_th = pa[:, 384:512]        # block-diag h->tau
        btp = pa[:, 512:512 + OH]               # packed lora_B^T
        h0 = pa[:, 512 + OH:512 + 2 * OH]       # packed hidden_B^T
        at2 = pa[:, 512 + 2 * OH:PW]            # packed (2*lora_A)^T

        tmin_sb = const.tile([128, 1], f32)
        nc.vector.memset(tmin_sb, TAU_MIN)

        # Pre-warm the Sigmoid/Exp activation tables while the param DMAs
        # are in flight: an ACT_TABLE_LOAD is 1.28us, and the liquid phase
        # only uses these two functions so the tables then stay resident.
        warm = scr.tile([1, 8], f32, tag="scr_w")
        nc.scalar.activation(out=warm[:, 0:1], in_=tmin_sb[0:1, 0:1],
                             func=AF.Sigmoid)
        nc.scalar.activation(out=warm[:, 1:2], in_=tmin_sb[0:1, 0:1],
                             func=AF.Exp)

        tt_all = const.tile([64, M_CORE], f16)   # stage-1 results
        beff = const.tile([64, O_], f16)         # unpacked B_eff^T

        # ---- liquid dynamics (replicated on every core) ---------------------
        # Packed [128, OH]: p<64 -> (r=p, o<OH), p>=64 -> (r=p-64, o>=OH).
        # Block-diagonal weights make each gate matmul 128-deep so one
        # matmul pair (target-part + h-part) covers both o-halves.
        # The 1/tau and 1/a reciprocals must run in f32 (DVE approx recip is
        # fp32-only; TT divide is not valid ISA), the rest of the chain is
        # fp16. TT inputs must share a dtype, so the f32->fp16 handoff is a
        # cast copy (ra16) and sigma(f) is materialized in both precisions.
        def liquid_step(step, h_cur):
            h_new = hpool.tile([128, OH], f16, tag="h", name=f"h{step}")
            sf32 = [None] * NCH
            sf16 = [None] * NCH
            s_u = [None] * NCH
            gg = [None] * NCH
            # all gate matmuls + sigmoids of the step first: one sigmoid
            # table load per step instead of one per chunk
            for ch in range(NCH):
                sf32[ch] = lqp.tile([128, CH], f32, tag="lq32",
                                    name=f"sf32_{step}_{ch}")
                sf16[ch] = lq16p.tile([128, CH], f16, tag="lq16",
                                      name=f"sf16_{step}_{ch}")
                s_u[ch] = lqp.tile([128, CH], f32, tag="lq32",
                                   name=f"su{step}_{ch}")
                for wi, (w_t, w_h, b_ap) in enumerate((
                    (w_gt, w_gh, bgd_ap),
                    (w_tt, w_th, btd_ap),
                )):
                    for j in range(CH // 512):
                        jsl = slice(ch * CH + j * 512, ch * CH + (j + 1) * 512)
                        osl = slice(j * 512, (j + 1) * 512)
                        pre = ps_pre.tile([128, 512], f32, tag="pre",
                                          name=f"pre{step}_{ch}_{wi}_{j}")
                        nc.tensor.matmul(pre, lhsT=w_t, rhs=btp[:, jsl],
                                         start=True, stop=False)
                        nc.tensor.matmul(pre, lhsT=w_h, rhs=h_cur[:, jsl],
                                         start=False, stop=True)
                        if wi == 0:
                            nc.scalar.activation(out=sf32[ch][:, osl], in_=pre,
                                                 func=AF.Sigmoid, bias=b_ap)
                            nc.scalar.activation(out=sf16[ch][:, osl], in_=pre,
                                                 func=AF.Sigmoid, bias=b_ap)
                        else:
                            nc.scalar.activation(out=s_u[ch][:, osl], in_=pre,
                                                 func=AF.Sigmoid, bias=b_ap)
            for ch in range(NCH):
                csl = slice(ch * CH, (ch + 1) * CH)
                gg[ch] = gp.tile([128, CH], f16, tag="g", name=f"g{step}_{ch}")
                nc.gpsimd.tensor_mul(gg[ch], sf16[ch], btp[:, csl])
            # per-chunk closed-form update chains (exp emitted inline; the
            # two exps are adjacent on the ACT queue so the exp table loads
            # once per step)
            for ch in range(NCH):
                csl = slice(ch * CH, (ch + 1) * CH)
                tau = lqp.tile([128, CH], f32, tag="lq32", name=f"tau{step}_{ch}")
                nc.vector.tensor_scalar(tau, s_u[ch], TAU_MAX - TAU_MIN,
                                        TAU_MIN, ALU.mult, ALU.add)
                rt = lqp.tile([128, CH], f32, tag="lq32", name=f"rt{step}_{ch}")
                nc.vector.reciprocal_approx_fast(out=rt, in_=tau)
                a = lqp.tile([128, CH], f32, tag="lq32", name=f"a{step}_{ch}")
                nc.vector.tensor_add(a, rt, sf32[ch])
                e = lq16p.tile([128, CH], f16, tag="lq16", name=f"e{step}_{ch}")
                nc.scalar.activation(out=e, in_=a, func=AF.Exp, scale=-DT_STEP)
                ra = lqp.tile([128, CH], f32, tag="lq32", name=f"ra{step}_{ch}")
                nc.vector.reciprocal_approx_fast(out=ra, in_=a)
                ra16 = lq16p.tile([128, CH], f16, tag="lq16",
                                  name=f"ra16_{step}_{ch}")
                nc.vector.tensor_copy(out=ra16, in_=ra)
                p_ = lq16p.tile([128, CH], f16, tag="lq16", name=f"p{step}_{ch}")
                nc.vector.tensor_mul(p_, ra16, gg[ch])
                d_ = lq16p.tile([128, CH], f16, tag="lq16", name=f"d{step}_{ch}")
                nc.vector.tensor_sub(d_, h_cur[:, csl], p_)
                de = lq16p.tile([128, CH], f16, tag="lq16", name=f"de{step}_{ch}")
                nc.vector.tensor_mul(de, d_, e)
                nc.vector.tensor_add(h_new[:, csl], de, p_)
            return h_new

        # ---- main pipeline stage 1: tt = (2A) @ x_blk^T ---------------------
        xt_sb = {}

        def in_dma(b):
            t = xtp.tile([128, DC * M_BLK], f16, tag="xt", name=f"xt{b}")
            nc.sync.dma_start(out=t, in_=xt[b, :, :])
            xt_sb[b] = t

        tt_ps = {}

        def in_mm(b, copy=True):
            t = xt_sb[b]
            tpt = ps_out.tile([128, 1024], f32, tag="op", name=f"ttps{b}")
            tt_ps[b] = tpt
            tp = tpt[0:64, 0:512]
            for c in range(DC):
                nc.tensor.matmul(
                    tp, lhsT=at2[:, c * R_:(c + 1) * R_],
                    rhs=t[:, c * M_BLK:(c + 1) * M_BLK],
                    start=(c == 0), stop=(c == DC - 1))
            if copy:
                tt_copy(b)

        def tt_copy(b):
            # DVE, and deferred past the liquid for early blocks: a tt copy
            # waits on the block's last matmul, and anything queued behind
            # it on DVE (the liquid chains) would stall with it
            nc.vector.tensor_copy(out=tt_all[:, b * M_BLK:(b + 1) * M_BLK],
                                  in_=tt_ps[b][0:64, 0:512])

        # ---- main pipeline stage 2: out = tt^T @ B_eff^T --------------------
        osb_cur = [None]

        def out_chain(b):
            for ms in range(M_BLK // 128):
                msg = b * (M_BLK // 128) + ms
                pair, tpos = divmod(msg, 2)
                if tpos == 0:
                    osb_cur[0] = outp.tile([128, 2, O_], f16, tag="osb",
                                           name=f"osb{pair}")
                o_sb = osb_cur[0]
                lhsT = tt_all[:, msg * 128:(msg + 1) * 128]
                for oc2 in range(O_ // 1024):
                    op = ps_out.tile([128, 1024], f32, tag="op",
                                     name=f"op{msg}_{oc2}")
                    nc.tensor.matmul(
                        op[:, 0:512], lhsT=lhsT,
                        rhs=beff[:, oc2 * 1024:oc2 * 1024 + 512],
                        start=True, stop=True)
                    nc.tensor.matmul(
                        op[:, 512:1024], lhsT=lhsT,
                        rhs=beff[:, oc2 * 1024 + 512:(oc2 + 1) * 1024],
                        start=True, stop=True)
                    dst = o_sb[:, tpos, oc2 * 1024:(oc2 + 1) * 1024]
                    # strictly alternate DVE/ACT so consecutive PSUM
                    # evictions overlap and never serialize on one engine
                    if (msg * 4 + oc2) % 2 == 0:
                        nc.vector.tensor_copy(out=dst, in_=op)
                    else:
                        nc.scalar.copy(out=dst, in_=op)
                if tpos == 1:
                    nc.scalar.dma_start(out=outv[pair], in_=o_sb)

        # ---- driver ---------------------------------------------------------
        # Emission order sets each engine's program order. Arrival times:
        # params ~11.5us (qAct), b0 ~18 (qSP), b1 ~23 (qAct), b2 ~29 (qSP),
        # b3 ~41 (qSP, WAR on b0's buffer). The PE queue interleaves liquid
        # steps with stage-1 blocks, then stage-2 as soon as beff is ready,
        # so most matmuls run before the DVFS throttle kicks in.
        in_dma(0)
        in_dma(1)
        h = h0
        h = liquid_step(0, h)
        h = liquid_step(1, h)
        in_mm(0, copy=False)
        in_dma(2)
        h = liquid_step(2, h)

        # unpack B_eff^T to [64, O]
        nc.vector.tensor_copy(out=beff[:, 0:OH], in_=h[0:64, :])
        nc.gpsimd.dma_start(out=beff[:, OH:O_], in_=h[64:128, :])
        absorb_s(beff[0:1, OH:OH + 1])

        tt_copy(0)
        in_mm(1, copy=False)
        tt_copy(1)
        in_dma(3)
        out_chain(0)
        in_mm(2)
        out_chain(1)
        in_mm(3)
        out_chain(2)
        out_chain(3)
    nc.finalize()
    return nc


def make_host_inputs(x, lora_A, lora_B, hidden_B, W_gate, b_gate, W_tau,
                     b_tau, n_cores=N_CORES):
    """Host-side sharding / layout prep. Returns the per-core in_maps."""
    f16 = np.float16
    x = np.asarray(x, np.float32).reshape(M_TOTAL, D_)

    WgT = np.asarray(W_gate, np.float32).T                   # [2r, r]
    WtT = np.asarray(W_tau, np.float32).T

    def blockdiag(w64):
        bd = np.zeros((128, 128), np.float32)
        bd[0:64, 0:64] = w64
        bd[64:128, 64:128] = w64
        return bd

    w_gt = blockdiag(WgT[:R_])
    w_gh = blockdiag(WgT[R_:])
    w_tt = blockdiag(WtT[:R_])
    w_th = blockdiag(WtT[R_:])

    BT = np.asarray(lora_B, np.float32).T                    # [r, O]
    btp = np.concatenate([BT[:, :OH], BT[:, OH:]], axis=0)   # [128, OH]
    hT = np.asarray(hidden_B, np.float32).T
    h0p = np.concatenate([hT[:, :OH], hT[:, OH:]], axis=0)
    at2 = (2.0 * np.asarray(lora_A, np.float32)).T           # [D, r]
    at2_pk = at2.reshape(DC, 128, R_).transpose(1, 0, 2).reshape(128, DC * R_)

    params_np = np.ascontiguousarray(np.concatenate(
        [w_gt, w_gh, w_tt, w_th, btp, h0p, at2_pk], axis=1).astype(f16))
    bg = np.asarray(b_gate, np.float32)
    bt = np.asarray(b_tau, np.float32)
    sparams_np = np.ascontiguousarray(
        np.stack([np.concatenate([bg, bg]), np.concatenate([bt, bt])],
                 axis=1).astype(np.float32))

    x16 = x.astype(f16)
    shared = dict(params=params_np, sparams=sparams_np)
    in_maps = []
    for c in range(n_cores):
        xc = x16[c * M_CORE:(c + 1) * M_CORE]
        xp = xc.reshape(NB, M_BLK, DC, 128).transpose(0, 3, 2, 1)
        m = dict(shared)
        m["xt"] = np.ascontiguousarray(xp.reshape(NB, 128, DC * M_BLK))
        in_maps.append(m)
    return in_maps


_NC_CACHE = {}


def kernel(x, lora_A, lora_B, hidden_B, W_gate, b_gate, W_tau, b_tau):
    from concourse.bass_utils import run_bass_kernel_spmd

    global LAST_RESULTS
    key = "main"
    if key not in _NC_CACHE:
        _NC_CACHE[key] = build_nc()
    nc = _NC_CACHE[key]

    in_maps = make_host_inputs(x, lora_A, lora_B, hidden_B,
                               W_gate, b_gate, W_tau, b_tau)
    res = run_bass_kernel_spmd(nc, in_maps, core_ids=list(range(N_CORES)))
    LAST_RESULTS = res
    outs = [np.asarray(res.results[c]["out"]) for c in range(N_CORES)]
    full = np.concatenate(outs, axis=0).astype(np.float32).reshape(B_, S_, O_)
    return np.ascontiguousarray(full)


# revision 25
# speedup vs baseline: 1.2472x; 1.0002x over previous
"""Trainium2 Bass kernel for nn_LiquidLoRALayer (fp16 pipeline).

Computation (forward only; see problem reference):
    hidden <- 3 liquid-dynamics steps on [O, r] state (target = lora_B)
    B_eff   = hidden (the straight-through trick is a numeric no-op)
    out     = (x @ (2*lora_A)^T) @ B_eff^T          # SCALING=2 folded into A

Sharding: data-parallel over the B*S=16384 rows across 8 cores (2048 rows
per core); all small parameters replicated.

The whole pipeline runs in fp16: the kernel is DMA-bound (x in + out out),
so fp16 I/O halves the HBM traffic vs f32 (~33.7MB/core -> ~94us roofline
at 360 GB/s). fp16 keeps ~11 bits of mantissa so the rel-err stays ~1e-3.

Key layout choices:
  * x is pre-packed on the host to [NB, 128, DC*M_BLK] fp16 so each block
    DMA is 32KB-contiguous per partition (max DMA efficiency) and the
    contraction dim d sits on partitions.
  * liquid state is packed [128, OH] (r x o-half stacked on partitions);
    the gate matmuls use 128-deep block-diagonal weights so one matmul
    pair covers both o-halves (half the PE work of a 64-deep split).
  * bulk DMAs ride the two HWDGE queues: x blocks on qSP (sync), outputs
    on qAct (scalar); no SWDGE software descriptors on the hot path.
"""

import numpy as np
from contextlib import ExitStack

# Problem shapes (hardcoded per spec).
B_, S_, D_, O_, R_ = 4, 4096, 4096, 4096, 64
N_CORES = 8
M_TOTAL = B_ * S_
M_CORE = M_TOTAL // N_CORES      # 2048 rows per core
M_BLK = 512
NB = M_CORE // M_BLK             # 4 row blocks per core
DC = D_ // 128                   # 32 contraction chunks
OH = O_ // 2                     # packed-half width
CH = 1024                        # liquid elementwise chunk
NCH = OH // CH

SCALING = 128.0 / 64.0
DT_STEP = 0.1
TAU_MIN = 0.1
TAU_MAX = 10.0
ADAPT_STEPS = 3

PW = 512 + 2 * OH + DC * R_      # param blob cols: 4 W tiles | btp | h0 | at2

LAST_RESULTS = None  # stashed BassKernelResults from the most recent run


def build_nc():
    """Build the per-core Bass program. All 8 cores run this same program
    on different `xt` shards."""
    import concourse.bacc as bacc
    import concourse.tile as tile
    import concourse.mybir as mybir

    f32 = mybir.dt.float32
    f16 = mybir.dt.float16
    AF = mybir.ActivationFunctionType
    ALU = mybir.AluOpType

    nc = bacc.Bacc()
    xt = nc.dram_tensor("xt", [NB, 128, DC * M_BLK], f16, kind="ExternalInput")
    params = nc.dram_tensor("params", [128, PW], f16, kind="ExternalInput")
    sparams = nc.dram_tensor("sparams", [128, 2], f32, kind="ExternalInput")
    out = nc.dram_tensor("out", [M_CORE, O_], f16, kind="ExternalOutput")

    # out rows grouped in pairs of 128-row subtiles: row = n*256 + t*128 + p
    outv = out[:, :].rearrange("(n t p) o -> n p t o", p=128, t=2)

    with tile.TileContext(nc) as tc, ExitStack() as ctx:
        const = ctx.enter_context(tc.tile_pool(name="const", bufs=1))
        lqp = ctx.enter_context(tc.tile_pool(name="lq", bufs=7))
        lq16p = ctx.enter_context(tc.tile_pool(name="lq16", bufs=5))
        gp = ctx.enter_context(tc.tile_pool(name="gp", bufs=2))
        hpool = ctx.enter_context(tc.tile_pool(name="hbuf", bufs=2))
        xtp = ctx.enter_context(tc.tile_pool(name="xtp", bufs=3))
        outp = ctx.enter_context(tc.tile_pool(name="outp", bufs=2))
        scr = ctx.enter_context(tc.tile_pool(name="scr", bufs=4))
        ps_pre = ctx.enter_context(tc.tile_pool(name="ps_pre", bufs=2, space="PSUM"))
        ps_out = ctx.enter_context(tc.tile_pool(name="ps_out", bufs=3, space="PSUM"))

        def absorb_s(ap):
            t = scr.tile([1, 8], f16, tag="scr_s")
            nc.scalar.copy(out=t[:, 0:1], in_=ap)

        # ---- params: tiny biases then the fp16 blob, heading qSP so the
        # liquid phase can start ~11us in; qAct stays free for output DMAs.
        spa = const.tile([128, 2], f32)
        nc.sync.dma_start(out=spa, in_=sparams[:, :])
        bgd_ap = spa[:, 0:1]
        btd_ap = spa[:, 1:2]

        pa = const.tile([128, PW], f16)
        nc.sync.dma_start(out=pa[:, 0:512 + 2 * OH], in_=params[:, 0:512 + 2 * OH])
        nc.sync.dma_start(out=pa[:, 512 + 2 * OH:PW], in_=params[:, 512 + 2 * OH:PW])
        w_gt = pa[:, 0:128]          # block-diag target->gate
        w_gh = pa[:, 128:256]        # block-diag h->gate
        w_tt = pa[:, 256:384]        # block-diag target->tau
        w_th = pa[:, 384:512]        # block-diag h->tau
        btp = pa[:, 512:512 + OH]               # packed lora_B^T
        h0 = pa[:, 512 + OH:512 + 2 * OH]       # packed hidden_B^T
        at2 = pa[:, 512 + 2 * OH:PW]            # packed (2*lora_A)^T

        tmin_sb = const.tile([128, 1], f32)
        nc.vector.memset(tmin_sb, TAU_MIN)

        # Pre-warm the Sigmoid/Exp activation tables while the param DMAs
        # are in flight: an ACT_TABLE_LOAD is 1.28us, and the liquid phase
        # only uses these two functions so the tables then stay resident.
        warm = scr.tile([1, 8], f32, tag="scr_w")
        nc.scalar.activation(out=warm[:, 0:1], in_=tmin_sb[0:1, 0:1],
                             func=AF.Sigmoid)
        nc.scalar.activation(out=warm[:, 1:2], in_=tmin_sb[0:1, 0:1],
                             func=AF.Exp)

        tt_all = const.tile([64, M_CORE], f16)   # stage-1 results
        beff = const.tile([64, O_], f16)         # unpacked B_eff^T

        # ---- liquid dynamics (replicated on every core) ---------------------
        # Packed [128, OH]: p<64 -> (r=p, o<OH), p>=64 -> (r=p-64, o>=OH).
        # Block-diagonal weights make each gate matmul 128-deep so one
        # matmul pair (target-part + h-part) covers both o-halves.
        # The 1/tau and 1/a reciprocals must run in f32 (DVE approx recip is
        # fp32-only; TT divide is not valid ISA), the rest of the chain is
        # fp16. TT inputs must share a dtype, so the f32->fp16 handoff is a
        # cast copy (ra16) and sigma(f) is materialized in both precisions.
        def liquid_step(step, h_cur):
            h_new = hpool.tile([128, OH], f16, tag="h", name=f"h{step}")
            sf32 = [None] * NCH
            sf16 = [None] * NCH
            s_u = [None] * NCH
            gg = [None] * NCH
            # all gate matmuls + sigmoids of the step first: one sigmoid
            # table load per step instead of one per chunk
            for ch in range(NCH):
                sf32[ch] = lqp.tile([128, CH], f32, tag="lq32",
                                    name=f"sf32_{step}_{ch}")
                sf16[ch] = lq16p.tile([128, CH], f16, tag="lq16",
                                      name=f"sf16_{step}_{ch}")
                s_u[ch] = lqp.tile([128, CH], f32, tag="lq32",
                                   name=f"su{step}_{ch}")
                for wi, (w_t, w_h, b_ap) in enumerate((
                    (w_gt, w_gh, bgd_ap),
                    (w_tt, w_th, btd_ap),
                )):
                    for j in range(CH // 512):
                        jsl = slice(ch * CH + j * 512, ch * CH + (j + 1) * 512)
                        osl = slice(j * 512, (j + 1) * 512)
                        pre = ps_pre.tile([128, 512], f32, tag="pre",
                                          name=f"pre{step}_{ch}_{wi}_{j}")
                        nc.tensor.matmul(pre, lhsT=w_t, rhs=btp[:, jsl],
                                         start=True, stop=False)
                        nc.tensor.matmul(pre, lhsT=w_h, rhs=h_cur[:, jsl],
                                         start=False, stop=True)
                        if wi == 0:
                            nc.scalar.activation(out=sf32[ch][:, osl], in_=pre,
                                                 func=AF.Sigmoid, bias=b_ap)
                            nc.scalar.activation(out=sf16[ch][:, osl], in_=pre,
                                                 func=AF.Sigmoid, bias=b_ap)
                        else:
                            nc.scalar.activation(out=s_u[ch][:, osl], in_=pre,
                                                 func=AF.Sigmoid, bias=b_ap)
            for ch in range(NCH):
                csl = slice(ch * CH, (ch + 1) * CH)
                gg[ch] = gp.tile([128, CH], f16, tag="g", name=f"g{step}_{ch}")
                nc.gpsimd.tensor_mul(gg[ch], sf16[ch], btp[:, csl])
            # per-chunk closed-form update chains (exp emitted inline; the
            # two exps are adjacent on the ACT queue so the exp table loads
            # once per step)
            for ch in range(NCH):
                csl = slice(ch * CH, (ch + 1) * CH)
                tau = lqp.tile([128, CH], f32, tag="lq32", name=f"tau{step}_{ch}")
                nc.vector.tensor_scalar(tau, s_u[ch], TAU_MAX - TAU_MIN,
                                        TAU_MIN, ALU.mult, ALU.add)
                rt = lqp.tile([128, CH], f32, tag="lq32", name=f"rt{step}_{ch}")
                nc.vector.reciprocal_approx_fast(out=rt, in_=tau)
                a = lqp.tile([128, CH], f32, tag="lq32", name=f"a{step}_{ch}")
                nc.vector.tensor_add(a, rt, sf32[ch])
                e = lq16p.tile([128, CH], f16, tag="lq16", name=f"e{step}_{ch}")
                nc.scalar.activation(out=e, in_=a, func=AF.Exp, scale=-DT_STEP)
                ra = lqp.tile([128, CH], f32, tag="lq32", name=f"ra{step}_{ch}")
                nc.vector.reciprocal_approx_fast(out=ra, in_=a)
                ra16 = lq16p.tile([128, CH], f16, tag="lq16",
                                  name=f"ra16_{step}_{ch}")
                nc.vector.tensor_copy(out=ra16, in_=ra)
                p_ = lq16p.tile([128, CH], f16, tag="lq16", name=f"p{step}_{ch}")
                nc.vector.tensor_mul(p_, ra16, gg[ch])
                d_ = lq16p.tile([128, CH], f16, tag="lq16", name=f"d{step}_{ch}")
                nc.vector.tensor_sub(d_, h_cur[:, csl], p_)
                de = lq16p.tile([128, CH], f16, tag="lq16", name=f"de{step}_{ch}")
                nc.vector.tensor_mul(de, d_, e)
                nc.vector.tensor_add(h_new[:, csl], de, p_)
            return h_new

        # ---- main pipeline stage 1: tt = (2A) @ x_blk^T ---------------------
        xt_sb = {}

        def in_dma(b):
            t = xtp.tile([128, DC * M_BLK], f16, tag="xt", name=f"xt{b}")
            nc.sync.dma_start(out=t, in_=xt[b, :, :])
            xt_sb[b] = t

        tt_ps = {}

        def in_mm(b, copy=True):
            t = xt_sb[b]
            tpt = ps_out.tile([128, 1024], f32, tag="op", name=f"ttps{b}")
            tt_ps[b] = tpt
            tp = tpt[0:64, 0:512]
            for c in range(DC):
                nc.tensor.matmul(
                    tp, lhsT=at2[:, c * R_:(c + 1) * R_],
                    rhs=t[:, c * M_BLK:(c + 1) * M_BLK],
                    start=(c == 0), stop=(c == DC - 1))
            if copy:
                tt_copy(b)

        def tt_copy(b):
            # DVE, and deferred past the liquid for early blocks: a tt copy
            # waits on the block's last matmul, and anything queued behind
            # it on DVE (the liquid chains) would stall with it
            nc.vector.tensor_copy(out=tt_all[:, b * M_BLK:(b + 1) * M_BLK],
                                  in_=tt_ps[b][0:64, 0:512])

        # ---- main pipeline stage 2: out = tt^T @ B_eff^T --------------------
        osb_cur = [None]

        def out_chain(b):
            for ms in range(M_BLK // 128):
                msg = b * (M_BLK // 128) + ms
                pair, tpos = divmod(msg, 2)
                if tpos == 0:
                    osb_cur[0] = outp.tile([128, 2, O_], f16, tag="osb",
                                           name=f"osb{pair}")
                o_sb = osb_cur[0]
                lhsT = tt_all[:, msg * 128:(msg + 1) * 128]
                for oc2 in range(O_ // 1024):
                    op = ps_out.tile([128, 1024], f32, tag="op",
                                     name=f"op{msg}_{oc2}")
                    nc.tensor.matmul(
                        op[:, 0:512], lhsT=lhsT,
                        rhs=beff[:, oc2 * 1024:oc2 * 1024 + 512],
                        start=True, stop=True)
                    nc.tensor.matmul(
                        op[:, 512:1024], lhsT=lhsT,
                        rhs=beff[:, oc2 * 1024 + 512:(oc2 + 1) * 1024],
                        start=True, stop=True)
                    dst = o_sb[:, tpos, oc2 * 1024:(oc2 + 1) * 1024]
                    # strictly alternate DVE/ACT so consecutive PSUM
                    # evictions overlap and never serialize on one engine
                    if (msg * 4 + oc2) % 2 == 0:
                        nc.vector.tensor_copy(out=dst, in_=op)
                    else:
                        nc.scalar.copy(out=dst, in_=op)
                if tpos == 1:
                    nc.scalar.dma_start(out=outv[pair], in_=o_sb)

        # ---- driver ---------------------------------------------------------
        # Emission order sets each engine's program order. Arrival times:
        # params ~11.5us (qAct), b0 ~18 (qSP), b1 ~23 (qAct), b2 ~29 (qSP),
        # b3 ~41 (qSP, WAR on b0's buffer). The PE queue interleaves liquid
        # steps with stage-1 blocks, then stage-2 as soon as beff is ready,
        # so most matmuls run before the DVFS throttle kicks in.
        in_dma(0)
        in_dma(1)
        h = h0
        h = liquid_step(0, h)
        h = liquid_step(1, h)
        in_mm(0, copy=False)
        in_dma(2)
        h = liquid_step(2, h)

        # unpack B_eff^T to [64, O]
        nc.vector.tensor_copy(out=beff[:, 0:OH], in_=h[0:64, :])
        nc.gpsimd.dma_start(out=beff[:, OH:O_], in_=h[64:128, :])
        absorb_s(beff[0:1, OH:OH + 1])

        tt_copy(0)
        in_mm(1, copy=False)
        tt_copy(1)
        in_dma(3)
        in_mm(2)
        in_mm(3)
        out_chain(0)
        out_chain(1)
        out_chain(2)
        out_chain(3)
    nc.finalize()
    return nc


def make_host_inputs(x, lora_A, lora_B, hidden_B, W_gate, b_gate, W_tau,
                     b_tau, n_cores=N_CORES):
    """Host-side sharding / layout prep. Returns the per-core in_maps."""
    f16 = np.float16
    x = np.asarray(x, np.float32).reshape(M_TOTAL, D_)

    WgT = np.asarray(W_gate, np.float32).T                   # [2r, r]
    WtT = np.asarray(W_tau, np.float32).T

    def blockdiag(w64):
        bd = np.zeros((128, 128), np.float32)
        bd[0:64, 0:64] = w64
        bd[64:128, 64:128] = w64
        return bd

    w_gt = blockdiag(WgT[:R_])
    w_gh = blockdiag(WgT[R_:])
    w_tt = blockdiag(WtT[:R_])
    w_th = blockdiag(WtT[R_:])

    BT = np.asarray(lora_B, np.float32).T                    # [r, O]
    btp = np.concatenate([BT[:, :OH], BT[:, OH:]], axis=0)   # [128, OH]
    hT = np.asarray(hidden_B, np.float32).T
    h0p = np.concatenate([hT[:, :OH], hT[:, OH:]], axis=0)
    at2 = (2.0 * np.asarray(lora_A, np.float32)).T           # [D, r]
    at2_pk = at2.reshape(DC, 128, R_).transpose(1, 0, 2).reshape(128, DC * R_)

    params_np = np.ascontiguousarray(np.concatenate(
        [w_gt, w_gh, w_tt, w_th, btp, h0p, at2_pk], axis=1).astype(f16))
    bg = np.asarray(b_gate, np.float32)
    bt = np.asarray(b_tau, np.float32)
    sparams_np = np.ascontiguousarray(
        np.stack([np.concatenate([bg, bg]), np.concatenate([bt, bt])],
                 axis=1).astype(np.float32))

    x16 = x.astype(f16)
    shared = dict(params=params_np, sparams=sparams_np)
    in_maps = []
    for c in range(n_cores):
        xc = x16[c * M_CORE:(c + 1) * M_CORE]
        xp = xc.reshape(NB, M_BLK, DC, 128).transpose(0, 3, 2, 1)
        m = dict(shared)
        m["xt"] = np.ascontiguousarray(xp.reshape(NB, 128, DC * M_BLK))
        in_maps.append(m)
    return in_maps


_NC_CACHE = {}


def kernel(x, lora_A, lora_B, hidden_B, W_gate, b_gate, W_tau, b_tau):
    from concourse.bass_utils import run_bass_kernel_spmd

    global LAST_RESULTS
    key = "main"
    if key not in _NC_CACHE:
        _NC_CACHE[key] = build_nc()
    nc = _NC_CACHE[key]

    in_maps = make_host_inputs(x, lora_A, lora_B, hidden_B,
                               W_gate, b_gate, W_tau, b_tau)
    res = run_bass_kernel_spmd(nc, in_maps, core_ids=list(range(N_CORES)))
    LAST_RESULTS = res
    outs = [np.asarray(res.results[c]["out"]) for c in range(N_CORES)]
    full = np.concatenate(outs, axis=0).astype(np.float32).reshape(B_, S_, O_)
    return np.ascontiguousarray(full)
